# revision 6
# baseline (speedup 1.0000x reference)
"""2-layer GCN on 8 TRN2 NeuronCores (bf16 message-passing pipeline).

Strategy (dst-sharded message passing, bf16 end-to-end):
  - Nodes are grouped into 392 windows of 128 dst nodes; windows are assigned
    to (core, slot) pairs, load-balanced so every core owns 49 slots.
  - norm folding: table rows are pre-scaled by dinv[src] (layer 1 on the
    host: x' = dinv*x; layer 2 by folding dinv into the layer-1 flush scale,
    x2' = relu(dinv^2*agg + dinv*b)), so the dense phase is a pure matmul
    with no per-tile scaling. The dst-side dinv lands in the flush
    activation's per-partition scale; the bias is seeded into PSUM as a
    sqrt(deg) (x) b outer-product matmul.
  - Per layer, each core: builds the full dense table g = x' @ W in bf16
    (redundantly, 8-window PSUM-bank batches with a single cast op per
    bank), writes it node-major to DRAM (256B rows: 64 real bf16 features +
    64 never-written pad cols, because InstDMAGatherAnt requires 256B-
    multiple rows), then gathers g[src] for its edges (two overlapping
    32768-row views dodge the int16 index limit) and scatter-adds 128-edge
    blocks into a PSUM accumulator with one-hot matmuls. One-hots are built
    on DVE with tensor_scalar(is_equal) against a resident iota tile (2x
    mode), and the bf16 one-hot is the 128-col stationary operand (FWL).
  - Between layers, the per-core [64, 6272] transposed activations are
    AllGathered (bf16) so every core can build the full layer-2 table.
"""
import numpy as np

N = 50000
E = 800000
F = 64
NCORES = 8
W = 128                    # dst window size
NSLOTS = 49                # slots (windows) per core
R = NSLOTS * W             # staged rows per core = 6272
NPAD = NCORES * R          # 50176
NWIN = NPAD // W           # 392
TBL = 32768                # gather table view rows (int16 limit)
HI_BASE = NPAD - TBL       # 17408
CHUNK = 4096               # gather idxs per instruction
NQ = 4                     # swdge queues
XB = 8                     # dense windows per PSUM-bank batch

_cache = {}


# ----------------------------------------------------------------- host prep

def _prep(edge_index):
    src = np.asarray(edge_index[0], dtype=np.int64)
    dst = np.asarray(edge_index[1], dtype=np.int64)

    deg = np.bincount(dst, minlength=N).astype(np.float64) + 1.0
    dinv = (1.0 / np.sqrt(deg)).astype(np.float32)
    sqd = np.sqrt(deg).astype(np.float32)

    # window sizes (real edges + self loops)
    wcnt = np.bincount(dst // W, minlength=NWIN)
    nreal_in_win = np.minimum(np.maximum(N - np.arange(NWIN) * W, 0), W)
    wtot = wcnt + nreal_in_win

    # assign windows to (core, slot): sort desc, boustrophedon over cores
    order = np.argsort(-wtot, kind="stable")
    core_of_w = np.empty(NWIN, np.int64)
    slot_of_w = np.empty(NWIN, np.int64)
    for r, w in enumerate(order):
        j = r // NCORES
        k = r % NCORES
        c = k if (j % 2 == 0) else NCORES - 1 - k
        core_of_w[w] = c
        slot_of_w[w] = j

    nodes = np.arange(N)
    wn = nodes // W
    stage_of_node = core_of_w[wn] * R + slot_of_w[wn] * W + (nodes % W)
    node_of_stage = np.full(NPAD, -1, np.int64)
    node_of_stage[stage_of_node] = nodes

    # edge lists incl self loops (dst side); src as staged index
    all_src = np.concatenate([src, nodes])
    all_dst = np.concatenate([dst, nodes])
    e_w = all_dst // W
    e_core = core_of_w[e_w]
    e_slot = slot_of_w[e_w]
    e_dl = all_dst % W
    e_sidx = stage_of_node[all_src]

    # per (core, slot) lo/hi needs
    cs = e_core * NSLOTS + e_slot
    must_lo = e_sidx < HI_BASE
    must_hi = e_sidx >= TBL
    ncs = NCORES * NSLOTS
    n_tot = np.bincount(cs, minlength=ncs).reshape(NCORES, NSLOTS)
    n_lo = np.bincount(cs[must_lo], minlength=ncs).reshape(NCORES, NSLOTS)
    n_hi = np.bincount(cs[must_hi], minlength=ncs).reshape(NCORES, NSLOTS)

    BL = np.maximum(np.ceil(n_lo / W).max(axis=0), 1).astype(np.int64)
    hi_need = np.maximum(n_hi, n_tot - BL[None, :] * W)
    BH = np.maximum(np.ceil(hi_need / W).max(axis=0), 1).astype(np.int64)

    EL = int(BL.sum() * W)
    EH = int(BH.sum() * W)

    # block -> slot map (shared across cores)
    blk_slot_lo = np.repeat(np.arange(NSLOTS), BL)
    blk_slot_hi = np.repeat(np.arange(NSLOTS), BH)

    # per-core streams
    idx_lo = np.zeros((NCORES, EL), np.int64)
    idx_hi = np.zeros((NCORES, EH), np.int64)
    dl_lo = np.full((NCORES, EL), -1.0, np.float32)
    dl_hi = np.full((NCORES, EH), -1.0, np.float32)
    off_lo = np.concatenate([[0], np.cumsum(BL)]) * W
    off_hi = np.concatenate([[0], np.cumsum(BH)]) * W

    sort_key = cs
    eorder = np.argsort(sort_key, kind="stable")
    bounds = np.searchsorted(sort_key[eorder], np.arange(ncs + 1))
    for c in range(NCORES):
        for j in range(NSLOTS):
            seg = eorder[bounds[c * NSLOTS + j]:bounds[c * NSLOTS + j + 1]]
            sidx = e_sidx[seg]
            dl = e_dl[seg]
            m_lo = sidx < HI_BASE
            m_hi = sidx >= TBL
            flex = ~m_lo & ~m_hi
            cap_lo = int(BL[j] * W)
            nlo = int(m_lo.sum())
            flex_idx = np.nonzero(flex)[0]
            n_flex_lo = min(len(flex_idx), cap_lo - nlo)
            lo_sel = np.concatenate([np.nonzero(m_lo)[0], flex_idx[:n_flex_lo]])
            hi_sel = np.concatenate([np.nonzero(m_hi)[0], flex_idx[n_flex_lo:]])
            assert len(lo_sel) <= cap_lo and len(hi_sel) <= BH[j] * W, (c, j)
            o = off_lo[j]
            idx_lo[c, o:o + len(lo_sel)] = sidx[lo_sel]
            dl_lo[c, o:o + len(lo_sel)] = dl[lo_sel]
            o = off_hi[j]
            idx_hi[c, o:o + len(hi_sel)] = sidx[hi_sel] - HI_BASE
            dl_hi[c, o:o + len(hi_sel)] = dl[hi_sel]

    def pack16(a):  # [*, L] int -> [128, L/16] int16 (16-partition wrap, x8)
        t = a.astype(np.int16).reshape(-1, 16).T
        return np.tile(t, (8, 1))

    # per-core aux tensors
    dinv_stage = np.zeros(NPAD, np.float32)
    sqd_stage = np.zeros(NPAD, np.float32)
    real = node_of_stage >= 0
    dinv_stage[real] = dinv[node_of_stage[real]]
    sqd_stage[real] = sqd[node_of_stage[real]]

    prof = dict(BL=BL, BH=BH, EL=EL, EH=EH,
                blk_slot_lo=blk_slot_lo, blk_slot_hi=blk_slot_hi)
    per_core = []
    for c in range(NCORES):
        dv = dinv_stage[c * R:(c + 1) * R].reshape(NSLOTS, W).T  # [128, 49]
        per_core.append(dict(
            idx_lo=pack16(idx_lo[c]),
            idx_hi=pack16(idx_hi[c]),
            dl_lo=np.ascontiguousarray(dl_lo[c].reshape(-1, 128).T),   # [128, nblk]
            dl_hi=np.ascontiguousarray(dl_hi[c].reshape(-1, 128).T),
            sqd_own=sqd_stage[c * R:(c + 1) * R].reshape(1, R),
            sc1_own=np.ascontiguousarray(dv * dv),                     # dinv^2
            sc2_own=np.ascontiguousarray(dv),                          # dinv
        ))
    shared = dict(
        dinv=dinv,
        stage_of_node=stage_of_node,
        node_of_stage=node_of_stage,
    )
    return prof, per_core, shared


# ------------------------------------------------------------- bass program

def build_program(prof, rep=1, n_devices=NCORES, with_ag=True, parts="dg"):
    import concourse.bacc as bacc
    import concourse.mybir as mybir
    import concourse.tile as tile

    BL, BH = prof["BL"], prof["BH"]
    blk_slot_lo, blk_slot_hi = prof["blk_slot_lo"], prof["blk_slot_hi"]
    EL, EH = prof["EL"], prof["EH"]
    NBL, NBH = EL // W, EH // W
    f32 = mybir.dt.float32
    bf16 = mybir.dt.bfloat16

    nc = bacc.Bacc("TRN2", target_bir_lowering=False, debug=False,
                   enable_asserts=True, num_devices=n_devices,
                   num_swdge_queues=NQ)

    def din(name, shape, dt=f32):
        return nc.dram_tensor(name, shape, dt, kind="ExternalInput").ap()

    xT = din("xT", [F, NPAD], bf16)
    sqd_own = din("sqd_own", [1, R], bf16)
    sc1_own = din("sc1_own", [W, NSLOTS])        # f32, layer-1 flush scale
    sc2_own = din("sc2_own", [W, NSLOTS])        # f32, layer-2 flush scale
    W1 = din("W1f", [F, F], bf16)
    W2 = din("W2f", [F, F], bf16)
    b1 = din("b1f", [1, F], bf16)
    b2 = din("b2f", [1, F], bf16)
    iota_in = din("iota", [W, W], bf16)
    ident_in = din("ident", [W, W], bf16)
    idx_lo = din("idx_lo", [128, EL // 16], mybir.dt.int16)
    idx_hi = din("idx_hi", [128, EH // 16], mybir.dt.int16)
    dl_lo_in = din("dl_lo", [128, NBL])          # f32 (tensor_scalar scalar)
    dl_hi_in = din("dl_hi", [128, NBH])
    out = nc.dram_tensor("out", [R, F], f32, kind="ExternalOutput").ap()

    with tile.TileContext(nc) as tc:
        with tc.tile_pool(name="dram", bufs=1, space="DRAM") as dram, \
             tc.tile_pool(name="const", bufs=1) as constp, \
             tc.tile_pool(name="sb", bufs=3) as sb, \
             tc.tile_pool(name="gch", bufs=6) as gchp, \
             tc.tile_pool(name="oh", bufs=8) as ohp:

            g1 = dram.tile([NPAD, 2 * F], bf16)   # cols F: pad to 256B rows
            g2 = dram.tile([NPAD, 2 * F], bf16)
            ag_shard = dram.tile([F, R], bf16)
            ag_full = dram.tile([NCORES * F, R], bf16)

            # resident constants
            iota_t = constp.tile([W, W], bf16)
            nc.sync.dma_start(iota_t[:], iota_in[:])
            ident_t = constp.tile([W, W], bf16)
            nc.sync.dma_start(ident_t[:], ident_in[:])
            w1_t = constp.tile([F, F], bf16)
            nc.sync.dma_start(w1_t[:], W1[:])
            w2_t = constp.tile([F, F], bf16)
            nc.sync.dma_start(w2_t[:], W2[:])
            b1_t = constp.tile([1, F], bf16)
            nc.sync.dma_start(b1_t[:], b1[:])
            b2_t = constp.tile([1, F], bf16)
            nc.sync.dma_start(b2_t[:], b2[:])
            sqd_own_t = constp.tile([1, R], bf16)
            nc.sync.dma_start(sqd_own_t[:], sqd_own[:])
            sc1_own_t = constp.tile([W, NSLOTS], f32)
            nc.sync.dma_start(sc1_own_t[:], sc1_own[:])
            sc2_own_t = constp.tile([W, NSLOTS], f32)
            nc.sync.dma_start(sc2_own_t[:], sc2_own[:])
            idx_lo_t = constp.tile([128, EL // 16], mybir.dt.int16)
            nc.sync.dma_start(idx_lo_t[:], idx_lo[:])
            idx_hi_t = constp.tile([128, EH // 16], mybir.dt.int16)
            nc.sync.dma_start(idx_hi_t[:], idx_hi[:])
            dl_lo_t = constp.tile([128, NBL], f32)
            nc.sync.dma_start(dl_lo_t[:], dl_lo_in[:])
            dl_hi_t = constp.tile([128, NBH], f32)
            nc.sync.dma_start(dl_hi_t[:], dl_hi_in[:])

            qn = [0]

            def dense_phase(src_kind, w_t, g_t, psum):
                # g[t] = x' @ W, bf16 table, node-major rows in DRAM
                for t0 in range(0, NWIN, XB):
                    xt = sb.tile([F, XB * W], bf16, tag="xt")
                    if src_kind == "xT":
                        nc.sync.dma_start(xt[:], xT[:, t0 * W:(t0 + XB) * W])
                    else:
                        # batch contiguous runs within a core block of ag_full
                        k = 0
                        while k < XB:
                            t = t0 + k
                            c2, j2 = t // NSLOTS, t % NSLOTS
                            rl = min(XB - k, NSLOTS - j2)
                            nc.sync.dma_start(
                                xt[:, k * W:(k + rl) * W],
                                ag_full[c2 * F:(c2 + 1) * F,
                                        j2 * W:(j2 + rl) * W])
                            k += rl
                    pban = psum.tile([W, XB, F], f32, tag="pban")  # one bank
                    for k in range(XB):
                        nc.tensor.matmul(pban[:, k, :],
                                         lhsT=xt[:, k * W:(k + 1) * W],
                                         rhs=w_t[:], start=True, stop=True)
                    gb = sb.tile([W, XB, F], bf16, tag="gb")
                    nc.vector.tensor_copy(gb[:], pban[:])
                    dst = g_t[t0 * W:(t0 + XB) * W, 0:F].rearrange(
                        "(k p) f -> p k f", p=W)
                    nc.sync.dma_start(dst, gb[:])

            def gather_phase(g_t, b_t, psum_big):
                # bias into psum: psum[slot] = sqrt(deg) outer b
                for j in range(NSLOTS):
                    nc.tensor.matmul(
                        psum_big[:, j, :],
                        lhsT=sqd_own_t[:, j * W:(j + 1) * W],
                        rhs=b_t[:], start=True, stop=False,
                        skip_group_check=True)
                # find global last block per slot for stop flags
                last_blk = {}
                for b, j in enumerate(blk_slot_lo):
                    last_blk[int(j)] = ("lo", b)
                for b, j in enumerate(blk_slot_hi):
                    last_blk[int(j)] = ("hi", b)

                for half, nblk, idx_t, dl_t, blk_slot, base in (
                        ("lo", NBL, idx_lo_t, dl_lo_t, blk_slot_lo, 0),
                        ("hi", NBH, idx_hi_t, dl_hi_t, blk_slot_hi, HI_BASE)):
                    view = g_t[base:base + TBL, :]
                    for c0 in range(0, nblk, CHUNK // W):
                        cb = min(CHUNK // W, nblk - c0)
                        nidx = cb * W
                        gch = gchp.tile([128, CHUNK // W, 2 * F], bf16, tag="gch")
                        nc.gpsimd.dma_gather(
                            gch[:, :cb, :], view,
                            idx_t[:, c0 * (W // 16):(c0 + cb) * (W // 16)],
                            nidx, nidx, 2 * F, single_packet=False,
                            queue_num=qn[0] % NQ)
                        qn[0] += 1
                        if "s" not in parts and parts != "dg":
                            continue
                        for b in range(c0, c0 + cb):
                            j = int(blk_slot[b])
                            oh = ohp.tile([W, W], bf16, tag="oh")
                            nc.vector.tensor_scalar(
                                out=oh[:], in0=iota_t[:],
                                scalar1=dl_t[:, b:b + 1], scalar2=None,
                                op0=mybir.AluOpType.is_equal)
                            stop = last_blk[j] == (half, b)
                            nc.tensor.matmul(
                                psum_big[:, j, :],
                                lhsT=oh[:], rhs=gch[:, b - c0, 0:F],
                                start=False, stop=stop,
                                skip_group_check=True)

            for _ in range(rep):
                if "d" in parts:
                    with tc.tile_pool(name="psd1", bufs=3, space="PSUM") as psum:
                        dense_phase("xT", w1_t, g1, psum)
                x2T = constp.tile([F, R], bf16, tag="x2T")
                if "g" not in parts:
                    nc.gpsimd.memset(x2T[:], 0.0)
                if "g" in parts:
                    with tc.tile_pool(name="psg1", bufs=1, space="PSUM") as psumg, \
                         tc.tile_pool(name="pst1", bufs=1, space="PSUM") as psumt:
                        psum_big = psumg.tile([W, NSLOTS, F], f32)
                        gather_phase(g1, b1_t, psum_big)
                        # flush: x2' = relu(dinv^2 * psum); transpose to [64, R]
                        for j in range(NSLOTS):
                            x2j = sb.tile([W, F], bf16, tag="x2j")
                            nc.scalar.activation(
                                x2j[:], psum_big[:, j, :],
                                mybir.ActivationFunctionType.Relu,
                                scale=sc1_own_t[:, j:j + 1])
                            pt = psumt.tile([F, W], bf16, tag="pt")
                            nc.tensor.transpose(pt[:], x2j[:], ident_t[:])
                            nc.vector.tensor_copy(x2T[:, j * W:(j + 1) * W], pt[:])
                if with_ag:
                    nc.sync.dma_start(ag_shard[:], x2T[:])
                    nc.gpsimd.collective_compute(
                        "AllGather", mybir.AluOpType.bypass,
                        replica_groups=[list(range(n_devices))],
                        ins=[ag_shard[:].opt()],
                        outs=[ag_full[:].opt()])
                else:
                    for c in range(NCORES):
                        nc.sync.dma_start(ag_full[c * F:(c + 1) * F, :], x2T[:])
                if "d" in parts:
                    with tc.tile_pool(name="psd2", bufs=3, space="PSUM") as psum:
                        dense_phase("ag", w2_t, g2, psum)
                if "g" not in parts:
                    continue
                with tc.tile_pool(name="psg2", bufs=1, space="PSUM") as psumg:
                    psum_big = psumg.tile([W, NSLOTS, F], f32)
                    gather_phase(g2, b2_t, psum_big)
                    out_sb = sb.tile([W, NSLOTS, F], f32, tag="out_sb")
                    for j in range(NSLOTS):
                        nc.scalar.activation(
                            out_sb[:, j, :], psum_big[:, j, :],
                            mybir.ActivationFunctionType.Relu,
                            scale=sc2_own_t[:, j:j + 1])
                    nc.sync.dma_start(
                        out[:].rearrange("(j p) f -> p j f", p=W), out_sb[:])

    nc.compile()
    return nc


def make_in_maps(x, W1, b1, W2, b2, prof, per_core, shared):
    import ml_dtypes
    bf = ml_dtypes.bfloat16

    xs = np.asarray(x, np.float32) * shared["dinv"][:, None]
    xT_full = np.zeros((F, NPAD), np.float32)
    st = shared["stage_of_node"]
    xT_full[:, st] = xs.T
    iota = np.tile(np.arange(W, dtype=np.float32), (W, 1))
    ident = np.eye(W, dtype=np.float32)
    common = dict(
        xT=xT_full.astype(bf),
        W1f=np.asarray(W1, np.float32).astype(bf),
        W2f=np.asarray(W2, np.float32).astype(bf),
        b1f=np.asarray(b1, np.float32).reshape(1, F).astype(bf),
        b2f=np.asarray(b2, np.float32).reshape(1, F).astype(bf),
        iota=iota.astype(bf), ident=ident.astype(bf),
    )
    maps = []
    for c in range(NCORES):
        pc = per_core[c]
        maps.append(dict(
            common,
            idx_lo=pc["idx_lo"], idx_hi=pc["idx_hi"],
            dl_lo=pc["dl_lo"], dl_hi=pc["dl_hi"],
            sqd_own=pc["sqd_own"].astype(bf),
            sc1_own=pc["sc1_own"], sc2_own=pc["sc2_own"],
        ))
    return maps


def kernel(x, edge_index, W1, b1, W2, b2):
    from concourse.bass_utils import run_bass_kernel_spmd

    key = "main"
    if key not in _cache:
        prof, per_core, shared = _prep(edge_index)
        nc = build_program(prof)
        _cache[key] = (nc, prof, per_core, shared)
    nc, prof, per_core, shared = _cache[key]

    in_maps = make_in_maps(x, W1, b1, W2, b2, prof, per_core, shared)
    res = None
    last_exc = None
    for _ in range(4):
        # A crashed prior kernel can leave the NRT exec unit unrecoverable;
        # the first retry resets it.
        try:
            res = run_bass_kernel_spmd(nc, in_maps, core_ids=list(range(NCORES)))
            break
        except Exception as e:  # noqa: BLE001
            last_exc = e
    if res is None:
        raise last_exc
    staged = np.concatenate([res.results[c]["out"] for c in range(NCORES)], axis=0)
    return staged[shared["stage_of_node"]]


# revision 15
# speedup vs baseline: 2.9309x; 2.9309x over previous
"""2-layer GCN on 8 TRN2 NeuronCores (bf16 message-passing pipeline).

Strategy (dst-sharded message passing, bf16 end-to-end):
  - Nodes are grouped into 392 windows of 128 dst nodes; windows are assigned
    to (core, slot) pairs, load-balanced so every core owns 49 slots.
  - norm folding: table rows are pre-scaled by dinv[src] (layer 1 on the
    host: x' = dinv*x; layer 2 by folding dinv into the layer-1 flush scale,
    x2' = relu(dinv^2*agg + dinv*b)), so the dense phase is a pure matmul
    with no per-tile scaling. The dst-side dinv lands in the flush
    activation's per-partition scale; the bias is seeded into PSUM as a
    sqrt(deg) (x) b outer-product matmul.
  - Per layer, each core: builds the full dense table g = x' @ W in bf16
    (redundantly, 8-window PSUM-bank batches with a single cast op per
    bank), writes it node-major to DRAM (256B rows: 64 real bf16 features +
    64 never-written pad cols, because InstDMAGatherAnt requires 256B-
    multiple rows), then gathers g[src] for its edges (two overlapping
    32768-row views dodge the int16 index limit) and scatter-adds 128-edge
    blocks into a PSUM accumulator with one-hot matmuls. One-hots are built
    on DVE with tensor_scalar(is_equal) against a resident iota tile (2x
    mode), and the bf16 one-hot is the 128-col stationary operand (FWL).
  - Between layers, the per-core [64, 6272] transposed activations are
    AllGathered (bf16) so every core can build the full layer-2 table.
"""
import numpy as np

N = 50000
E = 800000
F = 64
NCORES = 8
W = 128                    # dst window size
NSLOTS = 49                # slots (windows) per core
R = NSLOTS * W             # staged rows per core = 6272
NPAD = NCORES * R          # 50176
NWIN = NPAD // W           # 392
TBL = 32768                # gather table view rows (int16 limit)
HI_BASE = NPAD - TBL       # 17408
CHUNK = 4096               # gather idxs per instruction
NQ = 4                     # swdge queues
XB = 8                     # dense windows per PSUM-bank batch

_cache = {}


# ----------------------------------------------------------------- host prep

def _prep(edge_index):
    src = np.asarray(edge_index[0], dtype=np.int64)
    dst = np.asarray(edge_index[1], dtype=np.int64)

    deg = np.bincount(dst, minlength=N).astype(np.float64) + 1.0
    dinv = (1.0 / np.sqrt(deg)).astype(np.float32)
    sqd = np.sqrt(deg).astype(np.float32)

    # window sizes (real edges + self loops)
    wcnt = np.bincount(dst // W, minlength=NWIN)
    nreal_in_win = np.minimum(np.maximum(N - np.arange(NWIN) * W, 0), W)
    wtot = wcnt + nreal_in_win

    # assign windows to (core, slot): sort desc, boustrophedon over cores
    order = np.argsort(-wtot, kind="stable")
    core_of_w = np.empty(NWIN, np.int64)
    slot_of_w = np.empty(NWIN, np.int64)
    for r, w in enumerate(order):
        j = r // NCORES
        k = r % NCORES
        c = k if (j % 2 == 0) else NCORES - 1 - k
        core_of_w[w] = c
        slot_of_w[w] = j

    nodes = np.arange(N)
    wn = nodes // W
    stage_of_node = core_of_w[wn] * R + slot_of_w[wn] * W + (nodes % W)
    node_of_stage = np.full(NPAD, -1, np.int64)
    node_of_stage[stage_of_node] = nodes

    # edge lists incl self loops (dst side); src as staged index
    all_src = np.concatenate([src, nodes])
    all_dst = np.concatenate([dst, nodes])
    e_w = all_dst // W
    e_core = core_of_w[e_w]
    e_slot = slot_of_w[e_w]
    e_dl = all_dst % W
    e_sidx = stage_of_node[all_src]

    # per (core, slot) lo/hi needs
    cs = e_core * NSLOTS + e_slot
    must_lo = e_sidx < HI_BASE
    must_hi = e_sidx >= TBL
    ncs = NCORES * NSLOTS
    n_tot = np.bincount(cs, minlength=ncs).reshape(NCORES, NSLOTS)
    n_lo = np.bincount(cs[must_lo], minlength=ncs).reshape(NCORES, NSLOTS)
    n_hi = np.bincount(cs[must_hi], minlength=ncs).reshape(NCORES, NSLOTS)

    BL = np.maximum(np.ceil(n_lo / W).max(axis=0), 1).astype(np.int64)
    hi_need = np.maximum(n_hi, n_tot - BL[None, :] * W)
    BH = np.maximum(np.ceil(hi_need / W).max(axis=0), 1).astype(np.int64)

    EL = int(BL.sum() * W)
    EH = int(BH.sum() * W)

    # block -> slot map (shared across cores)
    blk_slot_lo = np.repeat(np.arange(NSLOTS), BL)
    blk_slot_hi = np.repeat(np.arange(NSLOTS), BH)

    # per-core streams
    idx_lo = np.zeros((NCORES, EL), np.int64)
    idx_hi = np.zeros((NCORES, EH), np.int64)
    dl_lo = np.full((NCORES, EL), -1.0, np.float32)
    dl_hi = np.full((NCORES, EH), -1.0, np.float32)
    off_lo = np.concatenate([[0], np.cumsum(BL)]) * W
    off_hi = np.concatenate([[0], np.cumsum(BH)]) * W

    sort_key = cs
    eorder = np.argsort(sort_key, kind="stable")
    bounds = np.searchsorted(sort_key[eorder], np.arange(ncs + 1))
    for c in range(NCORES):
        for j in range(NSLOTS):
            seg = eorder[bounds[c * NSLOTS + j]:bounds[c * NSLOTS + j + 1]]
            sidx = e_sidx[seg]
            dl = e_dl[seg]
            m_lo = sidx < HI_BASE
            m_hi = sidx >= TBL
            flex = ~m_lo & ~m_hi
            cap_lo = int(BL[j] * W)
            nlo = int(m_lo.sum())
            flex_idx = np.nonzero(flex)[0]
            n_flex_lo = min(len(flex_idx), cap_lo - nlo)
            lo_sel = np.concatenate([np.nonzero(m_lo)[0], flex_idx[:n_flex_lo]])
            hi_sel = np.concatenate([np.nonzero(m_hi)[0], flex_idx[n_flex_lo:]])
            assert len(lo_sel) <= cap_lo and len(hi_sel) <= BH[j] * W, (c, j)
            o = off_lo[j]
            idx_lo[c, o:o + len(lo_sel)] = sidx[lo_sel]
            dl_lo[c, o:o + len(lo_sel)] = dl[lo_sel]
            o = off_hi[j]
            idx_hi[c, o:o + len(hi_sel)] = sidx[hi_sel] - HI_BASE
            dl_hi[c, o:o + len(hi_sel)] = dl[hi_sel]

    def pack16(a):  # [*, L] int -> [128, L/16] int16 (16-partition wrap, x8)
        t = a.astype(np.int16).reshape(-1, 16).T
        return np.tile(t, (8, 1))

    # per-core aux tensors
    dinv_stage = np.zeros(NPAD, np.float32)
    sqd_stage = np.zeros(NPAD, np.float32)
    real = node_of_stage >= 0
    dinv_stage[real] = dinv[node_of_stage[real]]
    sqd_stage[real] = sqd[node_of_stage[real]]

    prof = dict(BL=BL, BH=BH, EL=EL, EH=EH,
                blk_slot_lo=blk_slot_lo, blk_slot_hi=blk_slot_hi)
    NBANK = (NSLOTS + 7) // 8
    per_core = []
    for c in range(NCORES):
        dv = dinv_stage[c * R:(c + 1) * R].reshape(NSLOTS, W).T  # [128, 49]
        sq = sqd_stage[c * R:(c + 1) * R].reshape(NSLOTS, W)     # [49, 128]
        sqd_bnk = np.zeros((8, NBANK * W), np.float32)
        for n in range(NBANK):
            kk = min(8, NSLOTS - n * 8)
            sqd_bnk[:kk, n * W:(n + 1) * W] = sq[n * 8:n * 8 + kk]
        per_core.append(dict(
            idx_lo=pack16(idx_lo[c]),
            idx_hi=pack16(idx_hi[c]),
            dl_lo=np.ascontiguousarray(dl_lo[c].reshape(-1, 128).T),   # [128, nblk]
            dl_hi=np.ascontiguousarray(dl_hi[c].reshape(-1, 128).T),
            sqd_bnk=sqd_bnk,
            sc1_own=np.ascontiguousarray(dv * dv),                     # dinv^2
            sc2_own=np.ascontiguousarray(dv),                          # dinv
        ))
    shared = dict(
        dinv=dinv,
        stage_of_node=stage_of_node,
        node_of_stage=node_of_stage,
    )
    return prof, per_core, shared


# ------------------------------------------------------------- bass program

def build_program(prof, rep=1, n_devices=NCORES, with_ag=True, parts="dg"):
    import concourse.bacc as bacc
    import concourse.mybir as mybir
    import concourse.tile as tile

    BL, BH = prof["BL"], prof["BH"]
    blk_slot_lo, blk_slot_hi = prof["blk_slot_lo"], prof["blk_slot_hi"]
    EL, EH = prof["EL"], prof["EH"]
    NBL, NBH = EL // W, EH // W
    f32 = mybir.dt.float32
    bf16 = mybir.dt.bfloat16

    nc = bacc.Bacc("TRN2", target_bir_lowering=False, debug=False,
                   enable_asserts=True, num_devices=n_devices,
                   num_swdge_queues=NQ)

    def din(name, shape, dt=f32):
        return nc.dram_tensor(name, shape, dt, kind="ExternalInput").ap()

    NBANK = (NSLOTS + 7) // 8
    CB = CHUNK // W
    xT = din("xT", [F, NPAD], bf16)
    sqd_bnk = din("sqd_bnk", [8, NBANK * W], bf16)
    sc1_own = din("sc1_own", [W, NSLOTS])        # f32, layer-1 flush scale
    sc2_own = din("sc2_own", [W, NSLOTS])        # f32, layer-2 flush scale
    W1 = din("W1f", [F, F], bf16)
    W2 = din("W2f", [F, F], bf16)
    bd1 = din("bd1", [8, 8 * F], bf16)           # block-diag bias (layer 1)
    bd2 = din("bd2", [8, 8 * F], bf16)
    iota_in = din("iota32", [W, CB * W], bf16)
    ident_in = din("ident", [W, W], bf16)
    idx_lo = din("idx_lo", [128, EL // 16], mybir.dt.int16)
    idx_hi = din("idx_hi", [128, EH // 16], mybir.dt.int16)
    dl_lo_in = din("dl_lo", [128, NBL], bf16)
    dl_hi_in = din("dl_hi", [128, NBH], bf16)
    out = nc.dram_tensor("out", [R, F], f32, kind="ExternalOutput").ap()

    with tile.TileContext(nc) as tc:
        with tc.tile_pool(name="dram", bufs=1, space="DRAM") as dram, \
             tc.tile_pool(name="const", bufs=1) as constp, \
             tc.tile_pool(name="sb", bufs=3) as sb, \
             tc.tile_pool(name="gch", bufs=6) as gchp, \
             tc.tile_pool(name="oh", bufs=3) as ohp:

            g1 = dram.tile([NPAD, 2 * F], bf16)   # cols F: pad to 256B rows
            g2 = dram.tile([NPAD, 2 * F], bf16)
            ag_shard = dram.tile([F, R], bf16)
            ag_full = dram.tile([NCORES * F, R], bf16)

            # resident constants
            iota_t = constp.tile([W, CB * W], bf16)
            nc.sync.dma_start(iota_t[:], iota_in[:])
            ident_t = constp.tile([W, W], bf16)
            nc.sync.dma_start(ident_t[:], ident_in[:])
            w1_t = constp.tile([F, F], bf16)
            nc.sync.dma_start(w1_t[:], W1[:])
            w2_t = constp.tile([F, F], bf16)
            nc.sync.dma_start(w2_t[:], W2[:])
            bd1_t = constp.tile([8, 8 * F], bf16)
            nc.sync.dma_start(bd1_t[:], bd1[:])
            bd2_t = constp.tile([8, 8 * F], bf16)
            nc.sync.dma_start(bd2_t[:], bd2[:])
            sqd_bnk_t = constp.tile([8, NBANK * W], bf16)
            nc.sync.dma_start(sqd_bnk_t[:], sqd_bnk[:])
            sc1_own_t = constp.tile([W, NSLOTS], f32)
            nc.sync.dma_start(sc1_own_t[:], sc1_own[:])
            sc2_own_t = constp.tile([W, NSLOTS], f32)
            nc.sync.dma_start(sc2_own_t[:], sc2_own[:])
            idx_lo_t = constp.tile([128, EL // 16], mybir.dt.int16)
            nc.sync.dma_start(idx_lo_t[:], idx_lo[:])
            idx_hi_t = constp.tile([128, EH // 16], mybir.dt.int16)
            nc.sync.dma_start(idx_hi_t[:], idx_hi[:])
            dl_lo_t = constp.tile([128, NBL], bf16)
            nc.sync.dma_start(dl_lo_t[:], dl_lo_in[:])
            dl_hi_t = constp.tile([128, NBH], bf16)
            nc.sync.dma_start(dl_hi_t[:], dl_hi_in[:])

            qn = [0]

            def dense_phase(src_kind, w_t, g_t, psum):
                # g[t] = x' @ W, bf16 table, node-major rows in DRAM
                for t0 in range(0, NWIN, XB):
                    xt = sb.tile([F, XB * W], bf16, tag="xt")
                    if src_kind == "xT":
                        nc.sync.dma_start(xt[:], xT[:, t0 * W:(t0 + XB) * W])
                    else:
                        # batch contiguous runs within a core block of ag_full
                        k = 0
                        while k < XB:
                            t = t0 + k
                            c2, j2 = t // NSLOTS, t % NSLOTS
                            rl = min(XB - k, NSLOTS - j2)
                            nc.sync.dma_start(
                                xt[:, k * W:(k + rl) * W],
                                ag_full[c2 * F:(c2 + 1) * F,
                                        j2 * W:(j2 + rl) * W])
                            k += rl
                    pban = psum.tile([W, XB, F], f32, tag="pban")  # one bank
                    for k in range(XB):
                        nc.tensor.matmul(pban[:, k, :],
                                         lhsT=xt[:, k * W:(k + 1) * W],
                                         rhs=w_t[:], start=True, stop=True)
                    gb = sb.tile([W, XB, F], bf16, tag="gb")
                    nc.vector.tensor_copy(gb[:], pban[:])
                    dst = g_t[t0 * W:(t0 + XB) * W, 0:F].rearrange(
                        "(k p) f -> p k f", p=W)
                    nc.sync.dma_start(dst, gb[:])

            def gather_phase(g_t, bd_t, psum_big):
                # bias into psum: one wide matmul per PSUM bank,
                # psum[d, 8n+k, f] = sqd_bnk[k, n*W+d] * b[f]
                for n in range(NBANK):
                    kk = min(8, NSLOTS - n * 8)
                    nc.tensor.matmul(
                        psum_big[:, n * 8:n * 8 + kk, :],
                        lhsT=sqd_bnk_t[0:kk, n * W:(n + 1) * W],
                        rhs=bd_t[0:kk, 0:kk * F], start=True, stop=False,
                        skip_group_check=True)
                # find global last block per slot for stop flags
                last_blk = {}
                for b, j in enumerate(blk_slot_lo):
                    last_blk[int(j)] = ("lo", b)
                for b, j in enumerate(blk_slot_hi):
                    last_blk[int(j)] = ("hi", b)

                for half, nblk, idx_t, dl_t, blk_slot, base in (
                        ("lo", NBL, idx_lo_t, dl_lo_t, blk_slot_lo, 0),
                        ("hi", NBH, idx_hi_t, dl_hi_t, blk_slot_hi, HI_BASE)):
                    view = g_t[base:base + TBL, :]
                    for c0 in range(0, nblk, CB):
                        cb = min(CB, nblk - c0)
                        nidx = cb * W
                        gch = gchp.tile([128, CB, 2 * F], bf16, tag="gch")
                        nc.gpsimd.dma_gather(
                            gch[:, :cb, :], view,
                            idx_t[:, c0 * (W // 16):(c0 + cb) * (W // 16)],
                            nidx, nidx, 2 * F, single_packet=False,
                            queue_num=qn[0] % NQ)
                        qn[0] += 1
                        if "s" not in parts and parts != "dg":
                            continue
                        oh = ohp.tile([W, CB, W], bf16, tag="oh")
                        nc.vector.tensor_tensor(
                            out=oh[:, :cb, :],
                            in0=dl_t[:, c0:c0 + cb].to_broadcast([W, cb, W]),
                            in1=iota_t[:, 0:cb * W].rearrange(
                                "p (c f) -> p c f", c=cb),
                            op=mybir.AluOpType.is_equal)
                        for b in range(c0, c0 + cb):
                            j = int(blk_slot[b])
                            stop = last_blk[j] == (half, b)
                            nc.tensor.matmul(
                                psum_big[:, j, :],
                                lhsT=oh[:, b - c0, :], rhs=gch[:, b - c0, 0:F],
                                start=False, stop=stop,
                                skip_group_check=True)

            for _ in range(rep):
                if "d" in parts:
                    with tc.tile_pool(name="psd1", bufs=3, space="PSUM") as psum:
                        dense_phase("xT", w1_t, g1, psum)
                x2T = constp.tile([F, R], bf16, tag="x2T")
                if "g" not in parts:
                    nc.gpsimd.memset(x2T[:], 0.0)
                if "g" in parts:
                    with tc.tile_pool(name="psg1", bufs=1, space="PSUM") as psumg, \
                         tc.tile_pool(name="pst1", bufs=1, space="PSUM") as psumt:
                        psum_big = psumg.tile([W, NSLOTS, F], f32)
                        gather_phase(g1, bd1_t, psum_big)
                        # flush: x2' = relu(dinv^2 * psum); transpose to [64, R]
                        for j in range(NSLOTS):
                            x2j = sb.tile([W, F], bf16, tag="x2j")
                            nc.scalar.activation(
                                x2j[:], psum_big[:, j, :],
                                mybir.ActivationFunctionType.Relu,
                                scale=sc1_own_t[:, j:j + 1])
                            pt = psumt.tile([F, W], bf16, tag="pt")
                            nc.tensor.transpose(pt[:], x2j[:], ident_t[:])
                            nc.vector.tensor_copy(x2T[:, j * W:(j + 1) * W], pt[:])
                if with_ag:
                    nc.sync.dma_start(ag_shard[:], x2T[:])
                    nc.gpsimd.collective_compute(
                        "AllGather", mybir.AluOpType.bypass,
                        replica_groups=[list(range(n_devices))],
                        ins=[ag_shard[:].opt()],
                        outs=[ag_full[:].opt()])
                else:
                    for c in range(NCORES):
                        nc.sync.dma_start(ag_full[c * F:(c + 1) * F, :], x2T[:])
                if "d" in parts:
                    with tc.tile_pool(name="psd2", bufs=3, space="PSUM") as psum:
                        dense_phase("ag", w2_t, g2, psum)
                if "g" not in parts:
                    continue
                with tc.tile_pool(name="psg2", bufs=1, space="PSUM") as psumg:
                    psum_big = psumg.tile([W, NSLOTS, F], f32)
                    gather_phase(g2, bd2_t, psum_big)
                    out_sb = sb.tile([W, NSLOTS, F], f32, tag="out_sb")
                    for j in range(NSLOTS):
                        nc.scalar.activation(
                            out_sb[:, j, :], psum_big[:, j, :],
                            mybir.ActivationFunctionType.Relu,
                            scale=sc2_own_t[:, j:j + 1])
                    nc.sync.dma_start(
                        out[:].rearrange("(j p) f -> p j f", p=W), out_sb[:])

    nc.compile()
    return nc


def make_in_maps(x, W1, b1, W2, b2, prof, per_core, shared):
    import ml_dtypes
    bf = ml_dtypes.bfloat16

    xs = np.asarray(x, np.float32) * shared["dinv"][:, None]
    xT_full = np.zeros((F, NPAD), np.float32)
    st = shared["stage_of_node"]
    xT_full[:, st] = xs.T
    CB = CHUNK // W
    iota32 = np.tile(np.arange(W, dtype=np.float32), (W, CB))
    ident = np.eye(W, dtype=np.float32)

    def bdiag(b):
        out = np.zeros((8, 8 * F), np.float32)
        bv = np.asarray(b, np.float32).reshape(F)
        for k in range(8):
            out[k, k * F:(k + 1) * F] = bv
        return out.astype(bf)

    common = dict(
        xT=xT_full.astype(bf),
        W1f=np.asarray(W1, np.float32).astype(bf),
        W2f=np.asarray(W2, np.float32).astype(bf),
        bd1=bdiag(b1), bd2=bdiag(b2),
        iota32=iota32.astype(bf), ident=ident.astype(bf),
    )
    maps = []
    for c in range(NCORES):
        pc = per_core[c]
        maps.append(dict(
            common,
            idx_lo=pc["idx_lo"], idx_hi=pc["idx_hi"],
            dl_lo=pc["dl_lo"].astype(bf), dl_hi=pc["dl_hi"].astype(bf),
            sqd_bnk=pc["sqd_bnk"].astype(bf),
            sc1_own=pc["sc1_own"], sc2_own=pc["sc2_own"],
        ))
    return maps


def kernel(x, edge_index, W1, b1, W2, b2):
    from concourse.bass_utils import run_bass_kernel_spmd

    key = "main"
    if key not in _cache:
        prof, per_core, shared = _prep(edge_index)
        nc = build_program(prof)
        _cache[key] = (nc, prof, per_core, shared)
    nc, prof, per_core, shared = _cache[key]

    in_maps = make_in_maps(x, W1, b1, W2, b2, prof, per_core, shared)
    res = None
    last_exc = None
    for _ in range(4):
        # A crashed prior kernel can leave the NRT exec unit unrecoverable;
        # the first retry resets it.
        try:
            res = run_bass_kernel_spmd(nc, in_maps, core_ids=list(range(NCORES)))
            break
        except Exception as e:  # noqa: BLE001
            last_exc = e
    if res is None:
        raise last_exc
    staged = np.concatenate([res.results[c]["out"] for c in range(NCORES)], axis=0)
    return staged[shared["stage_of_node"]]
